# revision 2
# baseline (speedup 1.0000x reference)
"""Multi-head causal attention (B=2, S=2048, H=16, Dh=64) on 8 TRN2 NeuronCores.

Sharding: tensor-parallel over heads - core c owns heads [2c, 2c+1] (a
128-wide feature block) for both batches; host sums the 8 partial
output projections.

v3 (from 251us baseline, via TimelineSim + hardware A/B bisection):
  - HW is PE-instruction-bound: each matmul instruction carries ~135ns
    of sequencer/weight-load overhead (measured by A/B: +384 small mms
    cost +52us). Kernel restructured to minimize PE instruction count
    (608 matmuls/iter) with big moving-free dims.
  - PSUM pools decoupled per phase: proj q/k/v/v-transpose rotate one
    bank, scores 2x2 banks, PV accumulators 2, oproj staging 1 = 8.
  - Causal mask: post-exp DVE multiply of the single triangular
    128-col strip only ([128,2,128] vs [128,2,512] full-area in the
    baseline); scores below the strip are never computed (col
    truncation) and the strip multiply is off the PE queue. -15us.
  - ACT runs ONLY the 80 exps (bisection: exp costs just ~2us of wall;
    out-DMA triggers that previously interleaved with exps on the ACT
    queue head-of-line-blocked them).
  - All on-chip attention tensors bf16 (q/k/v/p/a/wo): full-rate
    matmuls everywhere incl. truncated diagonal tiles (f32r pays 4x
    below 256 free), half SBUF/DVE traffic. rel err 4.2e-3 (gate 2e-2).
  - V||ones row-sum trick kept; ones written once outside the loop.
Measured (clean loop-slope harness, same-process A/B): 228-238us vs
baseline 247-253us. Session A/B notes (keep):
  - attention phase alone 147us; scores+exp 76; scores+pv-noexp 123;
    qkv chain (no DMA) 77; in-DMA 56; full 238. Walls are near-additive:
    phases barely overlap on HW.
  - REJECTED by A/B: 4-chunk pipelined q/k drains (+21us: instruction
    count beats pipelining); per-head score tiles bufs=4 (+12us);
    V projected direct-to-[j,m] via 32 small mms (+3..12us); out-DMA
    on SWDGE/Pool ring (+5us); PE additive mask matmuls (+15us vs DVE
    strip multiply).
  - GPSIMD cannot touch PSUM (BIR verifier); dma_start cannot read
    PSUM; matmul dtype rule: f32/f32r operands must match exactly.
"""

import numpy as np

import concourse.bass as bass
import concourse.mybir as mybir
import concourse.tile as tile
from concourse import bacc
from concourse.bass import ds
from concourse.masks import make_identity

B, S, H, Dh = 2, 2048, 16, 64
D = H * Dh            # 1024
NCORES = 8
HPC = H // NCORES     # heads per core = 2
M = HPC * Dh          # per-core feature block = 128
N = B * S             # 4096 token rows
IC = 512              # i-chunk (matmul moving free dim)
NICB = S // IC        # 4 i-chunks per batch
NJT = S // 128        # 16 j-tiles per batch

F32 = mybir.dt.float32
F32R = mybir.dt.float32r
BF16 = mybir.dt.bfloat16

XDT = BF16
AF = mybir.ActivationFunctionType
ALU = mybir.AluOpType

MASK_NEG = -1.0e30

# Timing-bisection flags (numerics are wrong unless all True); set e.g.
# kernel2.VARIANT["exp"] = False before _build_bass to drop the exps.
VARIANT = {"qkv": True, "scores": True, "exp": True, "pv": True,
           "oproj": True, "dma_in": True, "dma_out": True,
           "out_ring": "sp_act",  # or "pool": out DMA on SWDGE+SP rings
           "qkv_sub": False,      # drain q/k in 4 pipelined 128-col chunks
           "sc_per_head": False,  # [128,512] score tiles, bufs=4, exp/head
           "v_transpose": True,   # V via 8 big mms + 4 PE transposes
           "mask_dve": True}      # strip mask as DVE multiply (not PE mms)


def _build_bass(bench_iters=None):
    nc = bacc.Bacc("TRN2", target_bir_lowering=False, debug=False,
                   num_devices=NCORES)

    xqT = nc.dram_tensor("xqT", [D, N], XDT, kind="ExternalInput").ap()
    xkT = nc.dram_tensor("xkT", [D, N], XDT, kind="ExternalInput").ap()
    wqT = nc.dram_tensor("wqT", [D, M], XDT, kind="ExternalInput").ap()
    wkT = nc.dram_tensor("wkT", [D, M], XDT, kind="ExternalInput").ap()
    wvT = nc.dram_tensor("wvT", [D, M], XDT, kind="ExternalInput").ap()
    woT = nc.dram_tensor("woT", [M, D], BF16, kind="ExternalInput").ap()
    tri = nc.dram_tensor("tri", [128, 256], BF16, kind="ExternalInput").ap()
    out = nc.dram_tensor("out", [N, D], F32, kind="ExternalOutput").ap()

    with tile.TileContext(nc) as tc:
        with (
            tc.tile_pool(name="wts", bufs=1) as wpool,
            tc.tile_pool(name="xs", bufs=4) as xpool,
            tc.tile_pool(name="acts", bufs=2) as apool,
            tc.tile_pool(name="ps", bufs=6) as ppool,
            tc.tile_pool(name="proj", bufs=1, space="PSUM") as projps,
            tc.tile_pool(name="sc", bufs=2, space="PSUM") as scps,
            tc.tile_pool(name="pv", bufs=2, space="PSUM") as pvps,
            tc.tile_pool(name="op", bufs=1, space="PSUM") as opps,
        ):
            # --- constants ---
            wq_sb = wpool.tile([128, 8 * 128], XDT, tag="wq")
            wk_sb = wpool.tile([128, 8 * 128], XDT, tag="wk")
            wv_sb = wpool.tile([128, 8 * 128], XDT, tag="wv")
            wo_sb = wpool.tile([128, D], BF16, tag="wo")
            tri_sb = wpool.tile([128, 256], BF16, tag="tri")
            idn = wpool.tile([128, 128], BF16, tag="idn")
            # persistent V||ones tiles, one per batch parity
            vcs = [wpool.tile([128, NJT * HPC, 128], BF16, tag=f"vc{i}",
                              name=f"vc{i}")
                   for i in range(B)]
            nc.sync.dma_start(wq_sb.rearrange("p (c m) -> p c m", m=128),
                              wqT.rearrange("(c p) m -> p c m", p=128))
            nc.sync.dma_start(wk_sb.rearrange("p (c m) -> p c m", m=128),
                              wkT.rearrange("(c p) m -> p c m", p=128))
            nc.sync.dma_start(wv_sb.rearrange("p (c m) -> p c m", m=128),
                              wvT.rearrange("(c p) m -> p c m", p=128))
            nc.sync.dma_start(wo_sb[:], woT[:, :])
            nc.sync.dma_start(tri_sb[:], tri[:, :])
            make_identity(nc, idn[:])
            for vc in vcs:
                nc.gpsimd.memset(vc[:, :, 64:], 1.0)

            from contextlib import nullcontext
            loop_cm = (tc.For_i(0, bench_iters, 1)
                       if bench_iters else nullcontext())
            with loop_cm:
                _emit_body(nc, tc, locals())
    nc.finalize()
    return nc


def _emit_body(nc, tc, env):
    (xqT, xkT, out, wq_sb, wk_sb, wv_sb, wo_sb, tri_sb, idn, vcs,
     xpool, apool, ppool, projps, scps, pvps, opps) = (
        env["xqT"], env["xkT"], env["out"], env["wq_sb"], env["wk_sb"],
        env["wv_sb"], env["wo_sb"], env["tri_sb"], env["idn"], env["vcs"],
        env["xpool"], env["apool"], env["ppool"], env["projps"], env["scps"],
        env["pvps"], env["opps"])
    for b in range(B):
        kT = apool.tile([128, S], BF16, tag="kT")
        aT = apool.tile([128, S], BF16, tag="aT")
        vc = vcs[b]

        V = VARIANT
        for icb in range(NICB):
            i0 = b * S + icb * IC
            # ---- x DMA for this i-chunk (ring-spread) ----
            xq_t = xpool.tile([128, 8, IC], XDT, tag="xa")
            xk_t = xpool.tile([128, 8, IC], XDT, tag="xa")
            if V["dma_in"]:
                for g in range(4):
                    nc.sync.dma_start(
                        xq_t[:, ds(2 * g, 2), :],
                        xqT[ds(2 * g * 128, 256), ds(i0, IC)].rearrange(
                            "(c p) i -> p c i", p=128))
                    nc.scalar.dma_start(
                        xk_t[:, ds(2 * g, 2), :],
                        xkT[ds(2 * g * 128, 256), ds(i0, IC)].rearrange(
                            "(c p) i -> p c i", p=128))

            # ---- Q/K projections (shared PSUM slot, DVE copies out) ----
            qC = apool.tile([128, IC], BF16, tag="qC")
            if V["qkv"]:
                for which, w_sb, x_t in (("q", wq_sb, xq_t),
                                         ("k", wk_sb, xk_t)):
                    ps = projps.tile([128, IC], F32, tag="proj",
                                     name=f"ps_{b}_{icb}_{which}")
                    dst = (qC[:] if which == "q"
                           else kT[:, ds(icb * IC, IC)])
                    if V["qkv_sub"]:
                        # accumulate + drain in 4 column chunks so the DVE
                        # copies overlap the later chunks' matmuls
                        for cb in range(4):
                            for dc in range(8):
                                xop = (x_t[:, dc, ds(cb * 128, 128)]
                                       if V["dma_in"] else kT[:, ds(0, 128)])
                                nc.tensor.matmul(
                                    ps[:, ds(cb * 128, 128)],
                                    w_sb[:, ds(dc * 128, 128)], xop,
                                    start=(dc == 0), stop=(dc == 7))
                            nc.vector.tensor_copy(
                                dst[:, ds(cb * 128, 128)],
                                ps[:, ds(cb * 128, 128)])
                    else:
                        for dc in range(8):
                            xop = (x_t[:, dc, :] if V["dma_in"]
                                   else kT[:, ds(0, IC)])
                            nc.tensor.matmul(ps[:],
                                             w_sb[:, ds(dc * 128, 128)],
                                             xop,
                                             start=(dc == 0), stop=(dc == 7))
                        nc.vector.tensor_copy(dst, ps[:])

                if V["v_transpose"]:
                    # V like q/k (8 big mms, [m, j] layout), then 4 PE
                    # transposes + one strided DVE copy into the vc slots
                    vp = projps.tile([128, IC], F32, tag="proj",
                                     name=f"vp_{b}_{icb}")
                    for dc in range(8):
                        xop = (xk_t[:, dc, :] if V["dma_in"]
                               else kT[:, ds(0, IC)])
                        nc.tensor.matmul(vp[:], wv_sb[:, ds(dc * 128, 128)],
                                         xop,
                                         start=(dc == 0), stop=(dc == 7))
                    vt_sb = ppool.tile([128, IC], BF16, tag="vt")
                    nc.vector.tensor_copy(vt_sb[:], vp[:])
                    tp = projps.tile([128, IC], BF16, tag="proj",
                                     name=f"tp_{b}_{icb}")
                    for t in range(4):
                        nc.tensor.transpose(tp[:, ds(t * 128, 128)],
                                            vt_sb[:, ds(t * 128, 128)],
                                            idn[:])
                    nc.vector.tensor_copy(
                        vc[:, ds(icb * 4 * HPC, 4 * HPC), :64],
                        tp[:].rearrange("p (s c) -> p s c", c=64))
                else:
                    vp = projps.tile([128, 4, 128], F32, tag="proj",
                                     name=f"vp_{b}_{icb}")
                    for jb in range(4):
                        for dc in range(8):
                            xop = (xk_t[:, dc, ds(jb * 128, 128)]
                                   if V["dma_in"] else kT[:, ds(0, 128)])
                            nc.tensor.matmul(
                                vp[:, jb, :],
                                xop,
                                wv_sb[:, ds(dc * 128, 128)],
                                start=(dc == 0), stop=(dc == 7))
                        nc.vector.tensor_copy(
                            vc[:, ds((icb * 4 + jb) * HPC, HPC), :64],
                            vp[:, jb, :].rearrange("p (h m) -> p h m", m=64))

            # ---- causal attention for this i-chunk ----
            if not V["qkv"]:  # timing-only substitutes (always written)
                kT_s, qC_s, vc_s = wq_sb, wk_sb, None
            njt = 4 * icb + 4
            pv_tiles = [pvps.tile([128, IC], F32, tag="pv",
                                  name=f"pv_{b}_{icb}_{h}")
                        for h in range(HPC)]
            for jt in range(njt):
                rr = jt - 4 * icb
                toff = max(0, rr) * 128  # first needed col (causal)
                ncol = IC - toff
                if V["sc_per_head"]:
                    p_hs = []
                    for h in range(HPC):
                        s_ps = scps.tile([128, IC], F32, tag="sc", bufs=4,
                                         name=f"s_{b}_{icb}_{jt}_{h}")
                        if V["scores"]:
                            nc.tensor.matmul(
                                s_ps[:, toff:],
                                (kT[ds(h * 64, 64), ds(jt * 128, 128)]
                                 if V["qkv"] else
                                 wq_sb[ds(h * 64, 64),
                                       ds((jt % 8) * 128, 128)]),
                                (qC if V["qkv"] else wk_sb)[
                                    ds(h * 64, 64), toff:ncol + toff],
                                start=True, stop=(rr < 0))
                            if rr >= 0:
                                nc.tensor.matmul(
                                    s_ps[:, ds(toff, 128)],
                                    idn[:], tri_sb[:, ds(0, 128)],
                                    start=False, stop=True)
                        if V["exp"]:
                            p_h = ppool.tile([128, IC], BF16, tag="p",
                                             bufs=8,
                                             name=f"p_{b}_{icb}_{jt}_{h}")
                            src = (s_ps[:, toff:] if V["scores"]
                                   else xq_t[:, h, toff:])
                            nc.scalar.activation(p_h[:, toff:], src, AF.Exp)
                            p_hs.append(p_h)
                    if V["pv"]:
                        for h in range(HPC):
                            slot = jt * HPC + h
                            mv = (p_hs[h][:, toff:] if V["exp"] else
                                  (kT if V["qkv"] else wk_sb)[
                                      :, ds(toff, ncol)])
                            nc.tensor.matmul(
                                pv_tiles[h][:, toff:],
                                (vc[:, slot, :] if V["qkv"] else
                                 idn[:, :]),
                                mv,
                                start=(jt == 0), stop=(jt == njt - 1))
                else:
                    s_ps = scps.tile([128, HPC, IC], F32, tag="sc",
                                     name=f"s_{b}_{icb}_{jt}")
                    if V["scores"]:
                        for h in range(HPC):
                            nc.tensor.matmul(
                                s_ps[:, h, toff:],
                                (kT[ds(h * 64, 64), ds(jt * 128, 128)]
                                 if V["qkv"] else
                                 wq_sb[ds(h * 64, 64),
                                       ds((jt % 8) * 128, 128)]),
                                (qC if V["qkv"] else wk_sb)[
                                    ds(h * 64, 64), toff:ncol + toff],
                                start=True,
                                stop=(rr < 0 or V["mask_dve"]))
                            if rr >= 0 and not V["mask_dve"]:
                                # add -1e30 on the strictly-lower triangle
                                # of the single diagonal 128-col strip
                                nc.tensor.matmul(
                                    s_ps[:, h, ds(toff, 128)],
                                    idn[:], tri_sb[:, ds(0, 128)],
                                    start=False, stop=True)
                    if V["exp"]:
                        p_t = ppool.tile([128, HPC, IC], BF16, tag="p",
                                         name=f"p_{b}_{icb}_{jt}")
                        src = (s_ps[:, :, toff:] if V["scores"]
                               else xq_t[:, 0:HPC, toff:])
                        nc.scalar.activation(p_t[:, :, toff:], src, AF.Exp)
                        if rr >= 0 and V["mask_dve"]:
                            # zero the strictly-lower triangle of the
                            # diagonal 128-col strip (post-exp multiply)
                            nc.vector.tensor_tensor(
                                p_t[:, :, ds(toff, 128)],
                                p_t[:, :, ds(toff, 128)],
                                tri_sb[:, ds(128, 128)].unsqueeze(1)
                                .broadcast_to((128, HPC, 128)),
                                ALU.mult)
                    if V["pv"]:
                        for h in range(HPC):
                            slot = jt * HPC + h
                            mv = (p_t[:, h, toff:] if V["exp"] else
                                  (kT if V["qkv"] else wk_sb)[
                                      :, ds(toff, ncol)])
                            nc.tensor.matmul(
                                pv_tiles[h][:, toff:],
                                (vc[:, slot, :] if V["qkv"] else
                                 idn[:, :]),
                                mv,
                                start=(jt == 0), stop=(jt == njt - 1))
            for h in range(HPC):
                rc_t = ppool.tile([64, IC], F32, tag="rc")
                if V["pv"]:
                    nc.vector.reciprocal(rc_t[:], pv_tiles[h][ds(64, 64), :])
                    nc.vector.tensor_tensor(
                        aT[ds(h * 64, 64), ds(icb * IC, IC)],
                        pv_tiles[h][ds(0, 64), :],
                        rc_t[:], ALU.mult)

            # ---- partial out-projection for this i-chunk ----
            if V["oproj"]:
                for i128 in range(4):
                    ii = icb * 4 + i128
                    for dn in range(D // IC):
                        o_ps = opps.tile([128, IC], F32, tag="o")
                        o_stat = (aT[:, ds(ii * 128, 128)] if V["pv"]
                                  else kT[:, ds(i128 * 128, 128)])
                        nc.tensor.matmul(o_ps[:],
                                         o_stat,
                                         wo_sb[:, ds(dn * IC, IC)],
                                         start=True, stop=True)
                        o_sb = ppool.tile([128, IC], F32, tag="osb")
                        nc.vector.tensor_copy(o_sb[:], o_ps[:])
                        if V["dma_out"]:
                            if V["out_ring"] == "pool":
                                # late-dep out triggers stay off the ACT
                                # queue (they'd block exps in-order); 3/4
                                # on the SWDGE ring, 1/4 on SP
                                weng = (nc.sync if (ii + dn) % 4 == 1
                                        else nc.gpsimd)
                            else:
                                weng = (nc.sync if (ii + dn) % 2 == 0
                                        else nc.scalar)
                            weng.dma_start(
                                out[ds(b * S + ii * 128, 128),
                                    ds(dn * IC, IC)],
                                o_sb[:])


_STATE = {}


def _get_runner(bench_iters=None):
    """Build the Bass module and a cached jitted SPMD executor (compile once)."""
    global _STATE
    if bench_iters in _STATE:
        return _STATE[bench_iters]

    import jax
    from jax.sharding import Mesh, PartitionSpec
    from jax.experimental.shard_map import shard_map
    from concourse import bass2jax

    bass2jax.install_neuronx_cc_hook()
    nc = _build_bass(bench_iters)

    partition_name = (nc.partition_id_tensor.name
                      if nc.partition_id_tensor else None)
    in_names, out_names, out_avals, zero_shapes = [], [], [], []
    for alloc in nc.m.functions[0].allocations:
        if not isinstance(alloc, mybir.MemoryLocationSet):
            continue
        name = alloc.memorylocations[0].name
        if alloc.kind == "ExternalInput":
            if name != partition_name:
                in_names.append(name)
        elif alloc.kind == "ExternalOutput":
            shape = tuple(alloc.tensor_shape)
            dtype = mybir.dt.np(alloc.dtype)
            out_names.append(name)
            out_avals.append(jax.core.ShapedArray(shape, dtype))
            zero_shapes.append((shape, dtype))
    n_params = len(in_names)
    n_outs = len(out_avals)
    all_in_names = list(in_names) + list(out_names)
    if partition_name is not None:
        all_in_names.append(partition_name)

    def _body(*args):
        operands = list(args)
        if partition_name is not None:
            operands.append(bass2jax.partition_id_tensor())
        outs = bass2jax._bass_exec_p.bind(
            *operands,
            out_avals=tuple(out_avals),
            in_names=tuple(all_in_names),
            out_names=tuple(out_names),
            lowering_input_output_aliases=(),
            sim_require_finite=True,
            sim_require_nnan=True,
            nc=nc,
        )
        return tuple(outs)

    devices = jax.devices()[:NCORES]
    mesh = Mesh(np.asarray(devices), ("core",))
    in_specs = (PartitionSpec("core"),) * (n_params + n_outs)
    out_specs = (PartitionSpec("core"),) * n_outs
    donate = tuple(range(n_params, n_params + n_outs))
    sharded = jax.jit(
        shard_map(_body, mesh=mesh, in_specs=in_specs, out_specs=out_specs,
                  check_rep=False),
        donate_argnums=donate, keep_unused=True)

    def run(in_maps):
        concat_in = [
            np.concatenate([np.asarray(in_maps[c][k]) for c in range(NCORES)],
                           axis=0)
            for k in in_names
        ]
        concat_zeros = [np.zeros((NCORES * s[0], *s[1:]), dt)
                        for s, dt in zero_shapes]
        out_arrs = sharded(*concat_in, *concat_zeros)
        return [
            {k: np.asarray(out_arrs[i]).reshape(NCORES, *out_avals[i].shape)[c]
             for i, k in enumerate(out_names)}
            for c in range(NCORES)
        ]

    _STATE[bench_iters] = run
    return run


def _make_tri():
    """[128, 256]: cols 0:128 additive strip (0 / -1e30), 128:256
    multiplicative strip (1 / 0); allowed iff c >= j."""
    import ml_dtypes
    jj = np.arange(128)[:, None]
    cc = np.arange(128)[None, :]
    add = np.where(cc >= jj, 0.0, MASK_NEG)
    mul = np.where(cc >= jj, 1.0, 0.0)
    return np.concatenate([add, mul], axis=1).astype(ml_dtypes.bfloat16)


def prepare_in_maps(inputs_q, inputs_kv, Wq, Wk, Wv, Wo):
    import ml_dtypes
    xdt = ml_dtypes.bfloat16
    xq = np.ascontiguousarray(
        np.asarray(inputs_q, np.float32).reshape(N, D).T.astype(xdt))
    xk = np.ascontiguousarray(
        np.asarray(inputs_kv, np.float32).reshape(N, D).T.astype(xdt))
    Wq = np.asarray(Wq, np.float32)
    Wk = np.asarray(Wk, np.float32)
    Wv = np.asarray(Wv, np.float32)
    Wo = np.asarray(Wo, np.float32)
    tri = _make_tri()
    scale = 1.0 / np.sqrt(np.float32(Dh))
    in_maps = []
    for c in range(NCORES):
        sl = slice(c * M, (c + 1) * M)
        in_maps.append({
            "xqT": xq,
            "xkT": xk,
            "wqT": np.ascontiguousarray((Wq[sl, :] * scale).T.astype(xdt)),
            "wkT": np.ascontiguousarray(Wk[sl, :].T.astype(xdt)),
            "wvT": np.ascontiguousarray(Wv[sl, :].T.astype(xdt)),
            "woT": np.ascontiguousarray(Wo[:, sl].T.astype(xdt)),
            "tri": tri,
        })
    return in_maps


def _run_fallback(in_maps):
    """Slow-but-sure path: the stock SPMD runner (fresh compile per call)."""
    from concourse.bass_utils import run_bass_kernel_spmd
    nc = _build_bass()
    res = run_bass_kernel_spmd(nc, in_maps, core_ids=list(range(NCORES)))
    return res.results


def kernel(inputs_q, inputs_kv, mask, Wq, Wk, Wv, Wo):
    in_maps = prepare_in_maps(inputs_q, inputs_kv, Wq, Wk, Wv, Wo)
    try:
        results = _get_runner()(in_maps)
    except Exception:
        results = _run_fallback(in_maps)
    acc = results[0]["out"].astype(np.float32)
    for c in range(1, NCORES):
        acc = acc + results[c]["out"]
    return acc.reshape(B, S, D)
